# revision 3
# baseline (speedup 1.0000x reference)
"""Trainium2 Bass kernel for nn_MultiHeadAttention (B=2,T=2048,D=1024,H=16,HD=64).

Sharding: 8 cores = 2 batches x 4 heads/core (tensor parallel over heads).
Each core computes q,k,v projections for its 4 heads, RoPE, causal
flash-attention, and a partial output projection (its heads' slice of Wp);
the host sums the 4 partials per batch.

v3 design (on top of the fully-pipelined v2):
  - Startup: input DMAs split into column chunks and priority-ordered
    across the three DMA queues (sync-HW, scalar-HW, gpsimd-SW) so the
    first projection matmul starts as soon as wq + xT[:, :, 0:512] land
    (~6us of DMA) instead of after the whole 4.5MB input load.
  - exp activation table prefetched with a dummy 1-element exp at t=0.
  - Attention chunks processed in order 1..7 then 0, so the tail
    epilogue chain (asum drain -> den swap -> reciprocal -> normalize ->
    outproj -> store) hangs off the 2-iteration chunk 0 instead of the
    16-iteration chunk 7.
  - Zero-init matmuls for the PV accumulator removed: the first PV
    matmul of each psum bank uses start=True (clears the whole bank's
    has_written bits; the co-banked head's first matmul then overwrites
    since its bits are clear).
  - Epilogue split: asum bank-pair 0 drains on vector while pair 1
    drains on scalar; den swap is 2 combined DMAs; normalize muls split
    vector/gpsimd.
  - q/k RoPE'd tiles stored with lo/hi rows interleaved (one combined
    DMA per head instead of two): scores are invariant to any row
    permutation applied consistently to q and k.
  - Output stored bf16 (host accumulates partials in fp32).
"""

import os
import sys

sys.path.insert(0, "/opt/trn_rl_repo")

from contextlib import ExitStack

import numpy as np
import ml_dtypes

import concourse.bass as bass
import concourse.bacc as bacc
import concourse.tile as tile
import concourse.mybir as mybir
from concourse.bass import ts, ds
from concourse.bass_utils import run_bass_kernel_spmd

B, T, D, H, HD = 2, 2048, 1024, 16, 64
HPC = 4                # heads per core
E = HPC * HD           # 256 per-core channels
WP = 512               # projection chunk width (t)
WA = 256               # attention chunk width (q)
NPC = T // WP          # 4
NAC = T // WA          # 8
NKT = T // 128         # 16 k-tiles
DQ = D // 128          # 8 contraction subtiles
NEG = -10000.0
FP32 = mybir.dt.float32
BF16 = mybir.dt.bfloat16
SCALE = 1.0 / float(np.sqrt(HD))
NTT = T // 128         # 16 t-tiles for the output projection


def build_program(level=99):
    nc = bacc.Bacc("TRN2", target_bir_lowering=False, debug=False)
    xT_in = nc.declare_dram_parameter("xT_b", [D, T], BF16, isOutput=False)
    wqT = nc.declare_dram_parameter("wqT", [D, E], BF16, isOutput=False)
    wkT = nc.declare_dram_parameter("wkT", [D, E], BF16, isOutput=False)
    wvT = nc.declare_dram_parameter("wvT", [D, E], BF16, isOutput=False)
    wpT = nc.declare_dram_parameter("wpT", [E, D], BF16, isOutput=False)
    cosT = nc.declare_dram_parameter("cosT", [128, T], BF16, isOutput=False)
    sinT = nc.declare_dram_parameter("sinT", [128, T], BF16, isOutput=False)
    umask = nc.declare_dram_parameter("umask", [128, 128], BF16, isOutput=False)
    lmask = nc.declare_dram_parameter("lmask", [128, 640], BF16, isOutput=False)
    outp = nc.declare_dram_parameter("outp", [T, D], BF16, isOutput=True)

    with tile.TileContext(nc) as tc, ExitStack() as ctx:
        consts = ctx.enter_context(tc.tile_pool(name="consts", bufs=1))
        ropet = ctx.enter_context(tc.tile_pool(name="ropet", bufs=2))
        probs_p = ctx.enter_context(
            tc.tile_pool(name="probs", bufs=int(os.environ.get("K_PRBUFS", "2")))
        )
        asb_p = ctx.enter_context(tc.tile_pool(name="asb", bufs=2))
        den_p = ctx.enter_context(tc.tile_pool(name="den", bufs=2))
        ostage = ctx.enter_context(tc.tile_pool(name="ostage", bufs=2))
        warm_p = ctx.enter_context(tc.tile_pool(name="warm", bufs=1))
        ps_sc = ctx.enter_context(
            tc.tile_pool(
                name="ps_sc", bufs=int(os.environ.get("K_SCBUFS", "2")), space="PSUM"
            )
        )
        ps_acc = ctx.enter_context(tc.tile_pool(name="ps_acc", bufs=1, space="PSUM"))
        ps_io = ctx.enter_context(tc.tile_pool(name="ps_io", bufs=1, space="PSUM"))

        # ---- exp table prefetch: 1-element dummy activation at t~0 ----
        warm = warm_p.tile([1, 8], FP32, tag="warm")
        nc.vector.memset(warm[:, 0:4], 0.0)
        nc.scalar.activation(
            warm[:, 4:8], warm[:, 0:4], mybir.ActivationFunctionType.Exp, scale=1.0
        )

        # ---- constants / weights / x to SBUF, priority-ordered ----
        # Queue assignment (3 parallel DMA paths): sync=HW, scalar=HW,
        # gpsimd=SW.  Critical prefix: wq; xT j=0; wk; cos/sin j=0.
        xT_sb = consts.tile([128, DQ, T], BF16, tag="xT")
        xT_r = xT_in.rearrange("(o p) m -> p o m", p=128)
        wq_sb = consts.tile([128, DQ, E], BF16, tag="wq")
        wq_r = wqT.rearrange("(o p) m -> p o m", p=128)
        wk_sb = consts.tile([128, DQ, E], BF16, tag="wk")
        wk_r = wkT.rearrange("(o p) m -> p o m", p=128)
        wv_sb = consts.tile([128, DQ, E], BF16, tag="wv")
        wv_r = wvT.rearrange("(o p) m -> p o m", p=128)
        wp_sb = consts.tile([128, 2, D], BF16, tag="wp")
        cos_sb = consts.tile([128, T], BF16, tag="cos")
        sin_sb = consts.tile([128, T], BF16, tag="sin")
        u_sb = consts.tile([128, 128], BF16, tag="umask")
        lm_sb = consts.tile([128, 640], BF16, tag="lmask")

        # wave 0: wq (sync), xT j0 (scalar dq0-3 / gpsimd dq4-7)
        nc.sync.dma_start(wq_sb[:, 0:4, :], wq_r[:, 0:4, :])
        nc.sync.dma_start(wq_sb[:, 4:8, :], wq_r[:, 4:8, :])
        for dq in range(4):
            nc.scalar.dma_start(xT_sb[:, dq, ts(0, WP)], xT_r[:, dq, ts(0, WP)])
        for dq in range(4, 8):
            nc.gpsimd.dma_start(xT_sb[:, dq, ts(0, WP)], xT_r[:, dq, ts(0, WP)])
        # wave 1: wk (sync), cos/sin j0 (scalar), wv (gpsimd)
        nc.sync.dma_start(wk_sb[:, 0:4, :], wk_r[:, 0:4, :])
        nc.sync.dma_start(wk_sb[:, 4:8, :], wk_r[:, 4:8, :])
        nc.scalar.dma_start(cos_sb[:, ts(0, WP)], cosT[:, ts(0, WP)])
        nc.scalar.dma_start(sin_sb[:, ts(0, WP)], sinT[:, ts(0, WP)])
        nc.gpsimd.dma_start(wv_sb[:, 0:4, :], wv_r[:, 0:4, :])
        nc.gpsimd.dma_start(wv_sb[:, 4:8, :], wv_r[:, 4:8, :])
        # wave 2: xT j1 (sync), masks + cos/sin j1 (scalar), wp (gpsimd)
        for dq in range(DQ):
            nc.sync.dma_start(xT_sb[:, dq, ts(1, WP)], xT_r[:, dq, ts(1, WP)])
        nc.scalar.dma_start(u_sb[:], umask[:])
        nc.scalar.dma_start(lm_sb[:], lmask[:])
        nc.scalar.dma_start(cos_sb[:, ts(1, WP)], cosT[:, ts(1, WP)])
        nc.scalar.dma_start(sin_sb[:, ts(1, WP)], sinT[:, ts(1, WP)])
        nc.gpsimd.dma_start(wp_sb[:], wpT.rearrange("(o p) m -> p o m", p=128))
        # wave 3: xT j2 (sync/gpsimd), cos/sin j2-3 (scalar), xT j3
        for dq in range(0, DQ, 2):
            nc.sync.dma_start(xT_sb[:, dq, ts(2, WP)], xT_r[:, dq, ts(2, WP)])
            nc.gpsimd.dma_start(
                xT_sb[:, dq + 1, ts(2, WP)], xT_r[:, dq + 1, ts(2, WP)]
            )
        for j in (2, 3):
            nc.scalar.dma_start(cos_sb[:, ts(j, WP)], cosT[:, ts(j, WP)])
            nc.scalar.dma_start(sin_sb[:, ts(j, WP)], sinT[:, ts(j, WP)])
        for dq in range(0, DQ, 2):
            nc.sync.dma_start(xT_sb[:, dq, ts(3, WP)], xT_r[:, dq, ts(3, WP)])
            nc.gpsimd.dma_start(
                xT_sb[:, dq + 1, ts(3, WP)], xT_r[:, dq + 1, ts(3, WP)]
            )

        # per-head q/k tiles on partitions 0:64 — keeps every scores matmul
        # at PE tile_position (0,0); base-64 matmuls closing a group on a
        # partial psum bank crash the runtime.
        q_nat = [
            consts.tile([64, T], BF16, tag=f"qnat{h}", name=f"qnat{h}")
            for h in range(HPC)
        ]
        k_nat = [
            consts.tile([64, T], BF16, tag=f"knat{h}", name=f"knat{h}")
            for h in range(HPC)
        ]
        # v_aug[:, t, h, :]: even h = [v | ones], odd h = [ones | v]; fill
        # everything with ones, the v copies overwrite their halves.
        # memset on vector: DVE is idle during the initial DMA wait, and
        # this keeps the gpsimd engine free to issue its SW-DGE loads.
        v_aug = consts.tile([128, NKT, HPC, 128], BF16, tag="vaug")
        nc.vector.memset(v_aug[:], 1.0)
        attn_nrm = [
            consts.tile([128, T], BF16, tag=f"anrm{p}", name=f"anrm{p}")
            for p in range(2)
        ]

        # ---- work quanta (proj / outproj), drained between attn iters ----
        def emit_qk(j, w_sb, nat):
            pqk = ps_io.tile([128, 2, WP], FP32, tag="io", name="pqk")
            for half in range(2):
                for dq in range(DQ):
                    nc.tensor.matmul(
                        pqk[:, half, :],
                        lhsT=w_sb[:, dq, ds(128 * half, 128)],
                        rhs=xT_sb[:, dq, ts(j, WP)],
                        start=(dq == 0),
                        stop=(dq == DQ - 1),
                    )
            lo, hi = pqk[:, 0, :], pqk[:, 1, :]
            cs, sn = cos_sb[:, ts(j, WP)], sin_sb[:, ts(j, WP)]
            st = ropet.tile([128, 2, WP], BF16, tag="st", name="st")
            ta = ropet.tile([128, 2, WP], FP32, tag="ta", name="ta")
            tb = ropet.tile([128, 2, WP], FP32, tag="tb", name="tb")
            nc.vector.tensor_mul(ta[:, 0, :], lo, cs)
            nc.vector.tensor_mul(ta[:, 1, :], hi, sn)
            nc.vector.tensor_sub(st[:, 0, :], ta[:, 0, :], ta[:, 1, :])
            nc.vector.tensor_mul(tb[:, 0, :], hi, cs)
            nc.vector.tensor_mul(tb[:, 1, :], lo, sn)
            nc.vector.tensor_add(st[:, 1, :], tb[:, 0, :], tb[:, 1, :])
            # one combined DMA per head: rows land interleaved
            # (lo0,hi0,lo1,hi1,...) — scores are invariant to a row
            # permutation applied consistently to q and k.
            for h in range(HPC):
                nc.sync.dma_start(
                    nat[h][ds(0, 64), ts(j, WP)], st[ds(32 * h, 32), :, :]
                )

        def emit_v(j, half_pair):
            pv = ps_io.tile([128, 2, E], FP32, tag="io", name="pv")
            for tt in range(2):
                g = 4 * j + 2 * half_pair + tt
                for dq in range(DQ):
                    nc.tensor.matmul(
                        pv[:, tt, :],
                        lhsT=xT_sb[:, dq, ts(g, 128)],
                        rhs=wv_sb[:, dq, :],
                        start=(dq == 0),
                        stop=(dq == DQ - 1),
                    )
            for tt in range(2):
                g = 4 * j + 2 * half_pair + tt
                for h in range(HPC):
                    voff = 0 if h % 2 == 0 else 64
                    nc.vector.tensor_copy(
                        v_aug[:, g, h, ds(voff, 64)], pv[:, tt, ds(64 * h, 64)]
                    )

        def emit_po(g, pool=None, tag="io", tail=False):
            po = (pool or ps_io).tile([128, D], FP32, tag=tag, name="po")
            for dh in range(2):
                for p in range(2):
                    nc.tensor.matmul(
                        po[:, ds(512 * dh, 512)],
                        lhsT=attn_nrm[p][:, ts(g, 128)],
                        rhs=wp_sb[:, p, ds(512 * dh, 512)],
                        start=(p == 0),
                        stop=(p == 1),
                    )
            ost = ostage.tile([128, D], BF16, tag="ost", name="ost")
            if tail and g % 2 == 1:
                nc.scalar.copy(ost[:], po[:])
            else:
                nc.vector.tensor_copy(ost[:], po[:])
            if g % 2 == 0:
                nc.gpsimd.dma_start(outp[ts(g, 128), :], ost[:])
            else:
                nc.sync.dma_start(outp[ts(g, 128), :], ost[:])

        pending = []
        gap = [0]

        def drain_one():
            if pending and gap[0] >= 2:
                pending.pop(0)()
                gap[0] = 0

        # prologue: projection chunk 0 (serial; nothing to overlap with yet)
        if level >= 1 and not os.environ.get("K_NOPRO"):
            emit_qk(0, wq_sb, q_nat)
            emit_qk(0, wk_sb, k_nat)
            emit_v(0, 0)
            emit_v(0, 1)

        # chunk processing order: 1..7 then 0, so the tail epilogue chain
        # hangs off the tiny 2-iteration chunk 0.
        order = [1, 2, 3, 4, 5, 6, 7, 0]
        n_pos = {0: 0, 1: 0, 2: 1, 3: 4, 4: NAC}.get(level, NAC)
        if os.environ.get("K_NCHUNKS"):
            n_pos = int(os.environ["K_NCHUNKS"])
        # ---- attention chunks, with quanta interleaved ----
        for pos in range(n_pos):
            a = order[pos]
            if pos in (0, 2, 4) and level >= 3 and not os.environ.get("K_NOQUANTA"):
                j = pos // 2 + 1
                if j < NPC:
                    pending.append(lambda j=j: emit_qk(j, wq_sb, q_nat))
                    pending.append(lambda j=j: emit_qk(j, wk_sb, k_nat))
                    pending.append(lambda j=j: emit_v(j, 0))
                    pending.append(lambda j=j: emit_v(j, 1))
            if level >= 5:
                po_sched = {5: range(2, 8), 6: range(8, 14)}
                for g in po_sched.get(pos, ()):
                    pending.append(lambda g=g: emit_po(g))

            nk = 2 * a + 2
            asum = ps_acc.tile([128, HPC, WA], FP32, tag="acc", name="asum")

            def S(i, a=a):
                sct = ps_sc.tile([128, HPC, WA], FP32, tag="sc", name="sct")
                diag = i >= 2 * a
                for h in range(HPC):
                    nc.tensor.matmul(
                        sct[:, h, :],
                        lhsT=k_nat[h][:, ts(i, 128)],
                        rhs=q_nat[h][:, ts(a, WA)],
                        start=True,
                        stop=not diag,
                    )
                    if diag:
                        off = 384 - (128 * i - WA * a)
                        nc.tensor.matmul(
                            sct[:, h, :],
                            lhsT=u_sb[:],
                            rhs=lm_sb[:, ds(off, WA)],
                            start=False,
                            stop=True,
                        )
                return sct

            def EPV(i, sct, nk=nk, asum=asum):
                pr = probs_p.tile([128, HPC, WA], BF16, tag="pr", name="pr")
                nc.scalar.activation(
                    pr[:], sct[:], mybir.ActivationFunctionType.Exp, scale=SCALE
                )
                for h in range(HPC):
                    # i==0, even h: start=True clears the whole bank's
                    # has_written bits; the odd head's first matmul then
                    # overwrites (its bits are clear) — no zero-init needed.
                    nc.tensor.matmul(
                        asum[:, h, :],
                        lhsT=v_aug[:, i, h, :],
                        rhs=pr[:, h, :],
                        start=(i == 0 and h % 2 == 0),
                        stop=(i == nk - 1),
                        skip_group_check=True,
                    )

            # chunk 0 is processed last: emit the deferred outproj tiles for
            # chunk 7 right after chunk 0's first scores (they fill the PE
            # while chunk 0's exp runs).
            prev = None
            for i in range(nk):
                sct = S(i)
                if prev is not None:
                    EPV(prev[0], prev[1])
                prev = (i, sct)
                gap[0] += 1
                drain_one()
                if a == 0 and i == 1 and level >= 5:
                    emit_po(14)
                    emit_po(15)
            EPV(prev[0], prev[1])

            if os.environ.get("K_NOEPI"):
                continue
            # epilogue: drain asum bank-pair 0 on vector, pair 1 on scalar;
            # den swap = 2 combined DMAs; normalize muls split vector/gpsimd.
            asb = asb_p.tile([128, HPC, WA], FP32, tag="asb", name="asb")
            nc.vector.tensor_copy(asb[:, 0:2, :], asum[:, 0:2, :])
            nc.scalar.copy(asb[:, 2:4, :], asum[:, 2:4, :])
            den = den_p.tile([128, 2, WA], FP32, tag="den", name="den")
            nc.sync.dma_start(den[ds(0, 64), :, :], asb[ds(64, 64), 0:4:2, :])
            nc.sync.dma_start(den[ds(64, 64), :, :], asb[ds(0, 64), 1:4:2, :])
            rc = den_p.tile([128, 2, WA], FP32, tag="rc", name="rc")
            nc.vector.reciprocal_approx_fast(rc[:], den[:])
            nc.vector.tensor_mul(
                attn_nrm[0][ds(0, 64), ts(a, WA)],
                asb[ds(0, 64), 0, :],
                rc[ds(0, 64), 0, :],
            )
            nc.vector.tensor_mul(
                attn_nrm[0][ds(64, 64), ts(a, WA)],
                asb[ds(64, 64), 1, :],
                rc[ds(64, 64), 0, :],
            )
            nc.gpsimd.tensor_mul(
                attn_nrm[1][ds(0, 64), ts(a, WA)],
                asb[ds(0, 64), 2, :],
                rc[ds(0, 64), 1, :],
            )
            nc.gpsimd.tensor_mul(
                attn_nrm[1][ds(64, 64), ts(a, WA)],
                asb[ds(64, 64), 3, :],
                rc[ds(64, 64), 1, :],
            )

        # tail: whatever quanta remain + chunk 0's output tiles
        for f in pending:
            f()
        if level >= 5:
            # tail tiles go in the now-idle scores pool so they run in
            # parallel instead of serializing on the single-buffer io ring
            for g in range(2):
                emit_po(g, pool=ps_sc, tag="sc", tail=True)

    nc.compile()
    return nc


def make_consts(cos, sin):
    cosT = np.ascontiguousarray(
        np.tile(np.asarray(cos[0], dtype=np.float32).T[:32], (4, 1))
    ).astype(ml_dtypes.bfloat16)
    sinT = np.ascontiguousarray(
        np.tile(np.asarray(sin[0], dtype=np.float32).T[:32], (4, 1))
    ).astype(ml_dtypes.bfloat16)
    m = np.arange(128)[:, None]
    r = np.arange(128)[None, :]
    umask = np.where(r >= m, NEG, 0.0).astype(ml_dtypes.bfloat16)
    u_idx = np.arange(640)[None, :]
    lmask = (m >= u_idx - 383).astype(ml_dtypes.bfloat16)
    return dict(cosT=cosT, sinT=sinT, umask=umask, lmask=lmask)


def host_prep(core, xT_by_batch, Wq, Wk, Wv, Wp, consts):
    b, hp = core // 4, core % 4
    h0 = hp * HPC
    rows = slice(HD * h0, HD * h0 + E)
    Wq_s = np.asarray(Wq[rows]).reshape(HPC, HD, D)
    Wk_s = np.asarray(Wk[rows]).reshape(HPC, HD, D)
    wqT = np.ascontiguousarray(
        np.concatenate(
            [Wq_s[:, :32].reshape(128, D), Wq_s[:, 32:].reshape(128, D)], 0
        ).T.astype(ml_dtypes.bfloat16)
    )
    wkT = np.ascontiguousarray(
        np.concatenate(
            [Wk_s[:, :32].reshape(128, D), Wk_s[:, 32:].reshape(128, D)], 0
        ).T.astype(ml_dtypes.bfloat16)
    )
    wvT = np.ascontiguousarray(np.asarray(Wv[rows]).T.astype(ml_dtypes.bfloat16))
    wpT = np.ascontiguousarray(np.asarray(Wp[:, rows]).T.astype(ml_dtypes.bfloat16))
    return dict(
        xT_b=xT_by_batch[b],
        wqT=wqT,
        wkT=wkT,
        wvT=wvT,
        wpT=wpT,
        **consts,
    )


_NC_CACHE = None


def _get_nc():
    global _NC_CACHE
    if _NC_CACHE is None:
        _NC_CACHE = build_program()
    return _NC_CACHE


def kernel(x, cos, sin, Wq, Wk, Wv, Wp, _want_trace=False):
    x, cos, sin = np.asarray(x), np.asarray(cos), np.asarray(sin)
    Wq, Wk, Wv, Wp = (np.asarray(a) for a in (Wq, Wk, Wv, Wp))
    nc = _get_nc()
    consts = make_consts(cos, sin)
    xT_by_batch = [
        np.ascontiguousarray(x[b].T.astype(ml_dtypes.bfloat16)) for b in range(B)
    ]
    in_maps = [
        host_prep(core, xT_by_batch, Wq, Wk, Wv, Wp, consts) for core in range(8)
    ]
    res = run_bass_kernel_spmd(nc, in_maps, list(range(8)), trace=_want_trace)
    out = np.zeros((B, T, D), dtype=np.float32)
    for core in range(8):
        out[core // 4] += np.asarray(res.results[core]["outp"], dtype=np.float32)
    if _want_trace:
        kernel.last_exec_time_ns = res.exec_time_ns
        kernel.last_profile = res.profile_json
    return out


# revision 7
# speedup vs baseline: 1.1818x; 1.1818x over previous
"""Trainium2 Bass kernel for nn_MultiHeadAttention (B=2,T=2048,D=1024,H=16,HD=64).

Sharding: 8 cores = 2 batches x 4 heads/core (tensor parallel over heads).
Each core computes q,k,v projections for its 4 heads, RoPE, causal
flash-attention, and a partial output projection (its heads' slice of Wp);
the host sums the 4 partials per batch.

v3 design (on top of the fully-pipelined v2):
  - Startup: input DMAs split into column chunks and priority-ordered
    across the three DMA queues (sync-HW, scalar-HW, gpsimd-SW) so the
    first projection matmul starts as soon as wq + xT[:, :, 0:512] land
    (~6us of DMA) instead of after the whole 4.5MB input load.
  - exp activation table prefetched with a dummy 1-element exp at t=0.
  - Attention chunks processed in order 1..7 then 0, so the tail
    epilogue chain (asum drain -> den swap -> reciprocal -> normalize ->
    outproj -> store) hangs off the 2-iteration chunk 0 instead of the
    16-iteration chunk 7.
  - Zero-init matmuls for the PV accumulator removed: the first PV
    matmul of each psum bank uses start=True (clears the whole bank's
    has_written bits; the co-banked head's first matmul then overwrites
    since its bits are clear).
  - Epilogue split: asum bank-pair 0 drains on vector while pair 1
    drains on scalar; den swap is 2 combined DMAs; normalize muls split
    vector/gpsimd.
  - q/k RoPE'd tiles stored with lo/hi rows interleaved (one combined
    DMA per head instead of two): scores are invariant to any row
    permutation applied consistently to q and k.
  - Output stored bf16 (host accumulates partials in fp32).
"""

import os
import sys

sys.path.insert(0, "/opt/trn_rl_repo")

from contextlib import ExitStack

import numpy as np
import ml_dtypes

import concourse.bass as bass
import concourse.bacc as bacc
import concourse.tile as tile
import concourse.mybir as mybir
from concourse.bass import ts, ds
from concourse.bass_utils import run_bass_kernel_spmd

B, T, D, H, HD = 2, 2048, 1024, 16, 64
HPC = 4                # heads per core
E = HPC * HD           # 256 per-core channels
WP = 512               # projection chunk width (t)
WA = 256               # attention chunk width (q)
NPC = T // WP          # 4
NAC = T // WA          # 8
NKT = T // 128         # 16 k-tiles
DQ = D // 128          # 8 contraction subtiles
NEG = -10000.0
FP32 = mybir.dt.float32
BF16 = mybir.dt.bfloat16
SCALE = 1.0 / float(np.sqrt(HD))
NTT = T // 128         # 16 t-tiles for the output projection


def build_program(level=99):
    nc = bacc.Bacc("TRN2", target_bir_lowering=False, debug=False)
    xT_in = nc.declare_dram_parameter("xT_b", [D, T], BF16, isOutput=False)
    wqT = nc.declare_dram_parameter("wqT", [D, E], BF16, isOutput=False)
    wkT = nc.declare_dram_parameter("wkT", [D, E], BF16, isOutput=False)
    wvT = nc.declare_dram_parameter("wvT", [D, E], BF16, isOutput=False)
    wpT = nc.declare_dram_parameter("wpT", [E, D], BF16, isOutput=False)
    cosT = nc.declare_dram_parameter("cosT", [128, T], BF16, isOutput=False)
    sinT = nc.declare_dram_parameter("sinT", [128, T], BF16, isOutput=False)
    umask = nc.declare_dram_parameter("umask", [128, 128], BF16, isOutput=False)
    lmask = nc.declare_dram_parameter("lmask", [128, 640], BF16, isOutput=False)
    outp = nc.declare_dram_parameter("outp", [T, D], BF16, isOutput=True)

    with tile.TileContext(nc) as tc, ExitStack() as ctx:
        consts = ctx.enter_context(tc.tile_pool(name="consts", bufs=1))
        ropet = ctx.enter_context(tc.tile_pool(name="ropet", bufs=2))
        probs_p = ctx.enter_context(
            tc.tile_pool(name="probs", bufs=int(os.environ.get("K_PRBUFS", "2")))
        )
        asb_p = ctx.enter_context(tc.tile_pool(name="asb", bufs=2))
        den_p = ctx.enter_context(tc.tile_pool(name="den", bufs=2))
        ostage = ctx.enter_context(tc.tile_pool(name="ostage", bufs=2))
        warm_p = ctx.enter_context(tc.tile_pool(name="warm", bufs=1))
        ps_sc = ctx.enter_context(
            tc.tile_pool(
                name="ps_sc", bufs=int(os.environ.get("K_SCBUFS", "2")), space="PSUM"
            )
        )
        ps_acc = ctx.enter_context(tc.tile_pool(name="ps_acc", bufs=1, space="PSUM"))
        ps_io = ctx.enter_context(tc.tile_pool(name="ps_io", bufs=1, space="PSUM"))

        # ---- exp table prefetch: 1-element dummy activation at t~0 ----
        warm = warm_p.tile([1, 8], FP32, tag="warm")
        nc.vector.memset(warm[:, 0:4], 0.0)
        nc.scalar.activation(
            warm[:, 4:8], warm[:, 0:4], mybir.ActivationFunctionType.Exp, scale=1.0
        )

        # ---- constants / weights / x to SBUF, priority-ordered ----
        # Queue assignment (3 parallel DMA paths): sync=HW, scalar=HW,
        # gpsimd=SW.  Critical prefix: wq; xT j=0; wk; cos/sin j=0.
        xT_sb = consts.tile([128, DQ, T], BF16, tag="xT")
        xT_r = xT_in.rearrange("(o p) m -> p o m", p=128)
        wq_sb = consts.tile([128, DQ, E], BF16, tag="wq")
        wq_r = wqT.rearrange("(o p) m -> p o m", p=128)
        wk_sb = consts.tile([128, DQ, E], BF16, tag="wk")
        wk_r = wkT.rearrange("(o p) m -> p o m", p=128)
        wv_sb = consts.tile([128, DQ, E], BF16, tag="wv")
        wv_r = wvT.rearrange("(o p) m -> p o m", p=128)
        wp_sb = consts.tile([128, 2, D], BF16, tag="wp")
        cos_sb = consts.tile([128, T], BF16, tag="cos")
        sin_sb = consts.tile([128, T], BF16, tag="sin")
        u_sb = consts.tile([128, 128], BF16, tag="umask")
        lm_sb = consts.tile([128, 640], BF16, tag="lmask")

        # wave 0: wq (sync), xT j0 (scalar dq0-3 / gpsimd dq4-7)
        nc.sync.dma_start(wq_sb[:, 0:4, :], wq_r[:, 0:4, :])
        nc.sync.dma_start(wq_sb[:, 4:8, :], wq_r[:, 4:8, :])
        for dq in range(4):
            nc.scalar.dma_start(xT_sb[:, dq, ts(0, WP)], xT_r[:, dq, ts(0, WP)])
        for dq in range(4, 8):
            nc.gpsimd.dma_start(xT_sb[:, dq, ts(0, WP)], xT_r[:, dq, ts(0, WP)])
        # wave 1: wk (sync), cos/sin j0 (scalar), wv (gpsimd)
        nc.sync.dma_start(wk_sb[:, 0:4, :], wk_r[:, 0:4, :])
        nc.sync.dma_start(wk_sb[:, 4:8, :], wk_r[:, 4:8, :])
        nc.scalar.dma_start(cos_sb[:, ts(0, WP)], cosT[:, ts(0, WP)])
        nc.scalar.dma_start(sin_sb[:, ts(0, WP)], sinT[:, ts(0, WP)])
        nc.gpsimd.dma_start(wv_sb[:, 0:4, :], wv_r[:, 0:4, :])
        nc.gpsimd.dma_start(wv_sb[:, 4:8, :], wv_r[:, 4:8, :])
        # v_aug[:, t, h, :]: even h = [v | ones], odd h = [ones | v]; fill
        # everything with ones, the v copies overwrite their halves.
        # memset sits between gpsimd DMA waves: issued after wv, before wp,
        # so v_aug is ready by the first EPV without delaying the loads the
        # prologue needs.
        v_aug = consts.tile([128, NKT, HPC, 128], BF16, tag="vaug")
        nc.gpsimd.memset(v_aug[:], 1.0)

        # wave 2: xT j1 (sync), masks + cos/sin j1 (scalar), wp (gpsimd)
        for dq in range(DQ):
            nc.sync.dma_start(xT_sb[:, dq, ts(1, WP)], xT_r[:, dq, ts(1, WP)])
        nc.scalar.dma_start(u_sb[:], umask[:])
        nc.scalar.dma_start(lm_sb[:], lmask[:])
        nc.scalar.dma_start(cos_sb[:, ts(1, WP)], cosT[:, ts(1, WP)])
        nc.scalar.dma_start(sin_sb[:, ts(1, WP)], sinT[:, ts(1, WP)])
        nc.gpsimd.dma_start(wp_sb[:], wpT.rearrange("(o p) m -> p o m", p=128))
        # wave 3: xT j2 (sync/gpsimd), cos/sin j2-3 (scalar), xT j3
        for dq in range(0, DQ, 2):
            nc.sync.dma_start(xT_sb[:, dq, ts(2, WP)], xT_r[:, dq, ts(2, WP)])
            nc.gpsimd.dma_start(
                xT_sb[:, dq + 1, ts(2, WP)], xT_r[:, dq + 1, ts(2, WP)]
            )
        for j in (2, 3):
            nc.scalar.dma_start(cos_sb[:, ts(j, WP)], cosT[:, ts(j, WP)])
            nc.scalar.dma_start(sin_sb[:, ts(j, WP)], sinT[:, ts(j, WP)])
        for dq in range(0, DQ, 2):
            nc.sync.dma_start(xT_sb[:, dq, ts(3, WP)], xT_r[:, dq, ts(3, WP)])
            nc.gpsimd.dma_start(
                xT_sb[:, dq + 1, ts(3, WP)], xT_r[:, dq + 1, ts(3, WP)]
            )

        # per-head q/k tiles on partitions 0:64 — keeps every scores matmul
        # at PE tile_position (0,0); base-64 matmuls closing a group on a
        # partial psum bank crash the runtime.
        q_nat = [
            consts.tile([64, T], BF16, tag=f"qnat{h}", name=f"qnat{h}")
            for h in range(HPC)
        ]
        k_nat = [
            consts.tile([64, T], BF16, tag=f"knat{h}", name=f"knat{h}")
            for h in range(HPC)
        ]
        attn_nrm = [
            consts.tile([128, T], BF16, tag=f"anrm{p}", name=f"anrm{p}")
            for p in range(2)
        ]

        # ---- work quanta (proj / outproj), drained between attn iters ----
        def emit_qk(j, w_sb, nat):
            pqk = ps_io.tile([128, 2, WP], FP32, tag="io", name="pqk")
            for half in range(2):
                for dq in range(DQ):
                    nc.tensor.matmul(
                        pqk[:, half, :],
                        lhsT=w_sb[:, dq, ds(128 * half, 128)],
                        rhs=xT_sb[:, dq, ts(j, WP)],
                        start=(dq == 0),
                        stop=(dq == DQ - 1),
                    )
            lo, hi = pqk[:, 0, :], pqk[:, 1, :]
            cs, sn = cos_sb[:, ts(j, WP)], sin_sb[:, ts(j, WP)]
            st = ropet.tile([128, 2, WP], BF16, tag="st", name="st")
            ta = ropet.tile([128, 2, WP], FP32, tag="ta", name="ta")
            tb = ropet.tile([128, 2, WP], FP32, tag="tb", name="tb")
            nc.vector.tensor_mul(ta[:, 0, :], lo, cs)
            nc.vector.tensor_mul(ta[:, 1, :], hi, sn)
            nc.vector.tensor_sub(st[:, 0, :], ta[:, 0, :], ta[:, 1, :])
            nc.vector.tensor_mul(tb[:, 0, :], hi, cs)
            nc.vector.tensor_mul(tb[:, 1, :], lo, sn)
            nc.vector.tensor_add(st[:, 1, :], tb[:, 0, :], tb[:, 1, :])
            # one combined DMA per head: rows land interleaved
            # (lo0,hi0,lo1,hi1,...) — scores are invariant to a row
            # permutation applied consistently to q and k.
            for h in range(HPC):
                nc.sync.dma_start(
                    nat[h][ds(0, 64), ts(j, WP)], st[ds(32 * h, 32), :, :]
                )

        def emit_v(j, half_pair):
            pv = ps_io.tile([128, 2, E], FP32, tag="io", name="pv")
            for tt in range(2):
                g = 4 * j + 2 * half_pair + tt
                for dq in range(DQ):
                    nc.tensor.matmul(
                        pv[:, tt, :],
                        lhsT=xT_sb[:, dq, ts(g, 128)],
                        rhs=wv_sb[:, dq, :],
                        start=(dq == 0),
                        stop=(dq == DQ - 1),
                    )
            for tt in range(2):
                g = 4 * j + 2 * half_pair + tt
                for h in range(HPC):
                    voff = 0 if h % 2 == 0 else 64
                    nc.vector.tensor_copy(
                        v_aug[:, g, h, ds(voff, 64)], pv[:, tt, ds(64 * h, 64)]
                    )

        def emit_po(g, pool=None, tag="io", tail=False):
            po = (pool or ps_io).tile([128, D], FP32, tag=tag, name="po")
            for dh in range(2):
                for p in range(2):
                    nc.tensor.matmul(
                        po[:, ds(512 * dh, 512)],
                        lhsT=attn_nrm[p][:, ts(g, 128)],
                        rhs=wp_sb[:, p, ds(512 * dh, 512)],
                        start=(p == 0),
                        stop=(p == 1),
                    )
            ost = ostage.tile([128, D], BF16, tag="ost", name="ost")
            if tail and g % 2 == 1:
                nc.scalar.copy(ost[:], po[:])
            else:
                nc.vector.tensor_copy(ost[:], po[:])
            if g % 2 == 0:
                nc.gpsimd.dma_start(outp[ts(g, 128), :], ost[:])
            else:
                nc.sync.dma_start(outp[ts(g, 128), :], ost[:])

        pending = []
        gap = [0]

        def drain_one():
            if pending and gap[0] >= 2:
                pending.pop(0)()
                gap[0] = 0

        # prologue: projection chunk 0 (serial; nothing to overlap with yet)
        if level >= 1 and not os.environ.get("K_NOPRO"):
            emit_qk(0, wq_sb, q_nat)
            emit_qk(0, wk_sb, k_nat)
            emit_v(0, 0)
            emit_v(0, 1)

        # chunk processing order: 1..7 then 0, so the tail epilogue chain
        # hangs off the tiny 2-iteration chunk 0.
        order = [1, 2, 3, 4, 5, 6, 7, 0]
        n_pos = {0: 0, 1: 0, 2: 1, 3: 4, 4: NAC}.get(level, NAC)
        if os.environ.get("K_NCHUNKS"):
            n_pos = int(os.environ["K_NCHUNKS"])
        # ---- attention chunks, with quanta interleaved ----
        for pos in range(n_pos):
            a = order[pos]
            if pos in (0, 2, 4) and level >= 3 and not os.environ.get("K_NOQUANTA"):
                j = pos // 2 + 1
                if j < NPC:
                    pending.append(lambda j=j: emit_qk(j, wq_sb, q_nat))
                    pending.append(lambda j=j: emit_qk(j, wk_sb, k_nat))
                    pending.append(lambda j=j: emit_v(j, 0))
                    pending.append(lambda j=j: emit_v(j, 1))
            if level >= 5:
                po_sched = {5: range(2, 8), 6: range(8, 14)}
                for g in po_sched.get(pos, ()):
                    pending.append(lambda g=g: emit_po(g))

            nk = 2 * a + 2
            asum = ps_acc.tile([128, HPC, WA], FP32, tag="acc", name="asum")

            def S(i, a=a):
                sct = ps_sc.tile([128, HPC, WA], FP32, tag="sc", name="sct")
                diag = i >= 2 * a
                for h in range(HPC):
                    nc.tensor.matmul(
                        sct[:, h, :],
                        lhsT=k_nat[h][:, ts(i, 128)],
                        rhs=q_nat[h][:, ts(a, WA)],
                        start=True,
                        stop=not diag,
                    )
                    if diag:
                        off = 384 - (128 * i - WA * a)
                        nc.tensor.matmul(
                            sct[:, h, :],
                            lhsT=u_sb[:],
                            rhs=lm_sb[:, ds(off, WA)],
                            start=False,
                            stop=True,
                        )
                return sct

            def EPV(i, sct, nk=nk, asum=asum):
                pr = probs_p.tile([128, HPC, WA], BF16, tag="pr", name="pr")
                nc.scalar.activation(
                    pr[:], sct[:], mybir.ActivationFunctionType.Exp, scale=SCALE
                )
                for h in range(HPC):
                    # i==0, even h: start=True clears the whole bank's
                    # has_written bits; the odd head's first matmul then
                    # overwrites (its bits are clear) — no zero-init needed.
                    nc.tensor.matmul(
                        asum[:, h, :],
                        lhsT=v_aug[:, i, h, :],
                        rhs=pr[:, h, :],
                        start=(i == 0 and h % 2 == 0),
                        stop=(i == nk - 1),
                        skip_group_check=True,
                    )

            # chunk 0 is processed last: emit the deferred outproj tiles for
            # chunk 7 right after chunk 0's first scores (they fill the PE
            # while chunk 0's exp runs).
            prev = None
            for i in range(nk):
                sct = S(i)
                if prev is not None:
                    EPV(prev[0], prev[1])
                prev = (i, sct)
                gap[0] += 1
                drain_one()
                if a == 0 and i == 1 and level >= 5:
                    emit_po(14)
                    emit_po(15)
            EPV(prev[0], prev[1])

            if os.environ.get("K_NOEPI"):
                continue
            # epilogue: drain asum fast (vector), then normalize off the
            # critical path; den swap = 2 combined DMAs.
            asb = asb_p.tile([128, HPC, WA], FP32, tag="asb", name="asb")
            nc.vector.tensor_copy(asb[:], asum[:])
            den = den_p.tile([128, 2, WA], FP32, tag="den", name="den")
            nc.sync.dma_start(den[ds(0, 64), :, :], asb[ds(64, 64), 0:4:2, :])
            nc.sync.dma_start(den[ds(64, 64), :, :], asb[ds(0, 64), 1:4:2, :])
            rc = den_p.tile([128, 2, WA], FP32, tag="rc", name="rc")
            nc.vector.reciprocal_approx_fast(rc[:], den[:])
            for p in range(2):
                nc.vector.tensor_mul(
                    attn_nrm[p][ds(0, 64), ts(a, WA)],
                    asb[ds(0, 64), 2 * p, :],
                    rc[ds(0, 64), p, :],
                )
                nc.vector.tensor_mul(
                    attn_nrm[p][ds(64, 64), ts(a, WA)],
                    asb[ds(64, 64), 2 * p + 1, :],
                    rc[ds(64, 64), p, :],
                )

        # tail: whatever quanta remain + chunk 0's output tiles
        for f in pending:
            f()
        if level >= 5:
            # tail tiles go in the now-idle scores pool so they run in
            # parallel instead of serializing on the single-buffer io ring
            for g in range(2):
                emit_po(g, pool=ps_sc, tag="sc", tail=True)

    nc.compile()
    return nc


def make_consts(cos, sin):
    cosT = np.ascontiguousarray(
        np.tile(np.asarray(cos[0], dtype=np.float32).T[:32], (4, 1))
    ).astype(ml_dtypes.bfloat16)
    sinT = np.ascontiguousarray(
        np.tile(np.asarray(sin[0], dtype=np.float32).T[:32], (4, 1))
    ).astype(ml_dtypes.bfloat16)
    m = np.arange(128)[:, None]
    r = np.arange(128)[None, :]
    umask = np.where(r >= m, NEG, 0.0).astype(ml_dtypes.bfloat16)
    u_idx = np.arange(640)[None, :]
    lmask = (m >= u_idx - 383).astype(ml_dtypes.bfloat16)
    return dict(cosT=cosT, sinT=sinT, umask=umask, lmask=lmask)


def host_prep(core, xT_by_batch, Wq, Wk, Wv, Wp, consts):
    b, hp = core // 4, core % 4
    h0 = hp * HPC
    rows = slice(HD * h0, HD * h0 + E)
    Wq_s = np.asarray(Wq[rows]).reshape(HPC, HD, D)
    Wk_s = np.asarray(Wk[rows]).reshape(HPC, HD, D)
    wqT = np.ascontiguousarray(
        np.concatenate(
            [Wq_s[:, :32].reshape(128, D), Wq_s[:, 32:].reshape(128, D)], 0
        ).T.astype(ml_dtypes.bfloat16)
    )
    wkT = np.ascontiguousarray(
        np.concatenate(
            [Wk_s[:, :32].reshape(128, D), Wk_s[:, 32:].reshape(128, D)], 0
        ).T.astype(ml_dtypes.bfloat16)
    )
    wvT = np.ascontiguousarray(np.asarray(Wv[rows]).T.astype(ml_dtypes.bfloat16))
    wpT = np.ascontiguousarray(np.asarray(Wp[:, rows]).T.astype(ml_dtypes.bfloat16))
    return dict(
        xT_b=xT_by_batch[b],
        wqT=wqT,
        wkT=wkT,
        wvT=wvT,
        wpT=wpT,
        **consts,
    )


_NC_CACHE = None


def _get_nc():
    global _NC_CACHE
    if _NC_CACHE is None:
        _NC_CACHE = build_program()
    return _NC_CACHE


def kernel(x, cos, sin, Wq, Wk, Wv, Wp, _want_trace=False):
    x, cos, sin = np.asarray(x), np.asarray(cos), np.asarray(sin)
    Wq, Wk, Wv, Wp = (np.asarray(a) for a in (Wq, Wk, Wv, Wp))
    nc = _get_nc()
    consts = make_consts(cos, sin)
    xT_by_batch = [
        np.ascontiguousarray(x[b].T.astype(ml_dtypes.bfloat16)) for b in range(B)
    ]
    in_maps = [
        host_prep(core, xT_by_batch, Wq, Wk, Wv, Wp, consts) for core in range(8)
    ]
    res = run_bass_kernel_spmd(nc, in_maps, list(range(8)), trace=_want_trace)
    out = np.zeros((B, T, D), dtype=np.float32)
    for core in range(8):
        out[core // 4] += np.asarray(res.results[core]["outp"], dtype=np.float32)
    if _want_trace:
        kernel.last_exec_time_ns = res.exec_time_ns
        kernel.last_profile = res.profile_json
    return out
